# revision 1
# baseline (speedup 1.0000x reference)
"""Trainium2 Bass kernel for nn_BiasedMultiHeadAtten (8-core SPMD, tensor
parallel over heads).

The torch module's transpose(0,1)+reshape "scramble" means head n of the
attention only reads rows [64n,64n+64) u [1024+64n,1024+64n+64) of q/k, and
the per-head attention factors into four 1024x1024 score blocks with
contraction 64.  Sharding 2 heads per core therefore also shards the q/k
projections 8-way (256 of 2048 rows each).

Per core c (heads 2c, 2c+1):
  - project q,k for its 256 rows (contraction 4096, bf16 PE, fp32 psum)
  - scrambled attention: S^T = Y^T X per (a0,b0,b1-block), exp on ACT
    (no max subtraction: |scores| <= ~11), AV via PE with a ones-column
    appended to V^T producing the softmax denominators for free
  - out-proj partial: o_cols @ Wo[:,cols]^T  (full 2048 rows)
  - gated-residual branch for its 256 rows, added into the two row-tiles
    that the host-side feature permutation pins at tile 0/1
Host sums the 8 partial outputs with per-core row un-permutation.
"""

import numpy as np
import ml_dtypes

import concourse.bacc as bacc
import concourse.mybir as mybir
import concourse.tile as tile
from concourse import bass_utils

N_CORES = 8
L, H, E, E2, HD = 2048, 1024, 4096, 2048, 64
F32 = mybir.dt.float32
F16 = mybir.dt.float16
BF16 = mybir.dt.bfloat16
AF = mybir.ActivationFunctionType

_NC_CACHE = {}


def _perm16(c):
    """Block permutation: device l-tile j holds global l-tile perm[j];
    perm[0] = c and perm[1] = 8 + c so the residual rows sit at tiles 0,1."""
    perm = list(range(16))

    def place(pos, val):
        i = perm.index(val)
        perm[pos], perm[i] = perm[i], perm[pos]

    place(0, c)
    place(1, 8 + c)
    return perm


def _emit(nc, tc, d, out):
    from contextlib import ExitStack

    with ExitStack() as ctx:
        pers = ctx.enter_context(tc.tile_pool(name="pers", bufs=1))

        Y = [pers.tile([128, H], BF16, tag=f"Y{b}", name=f"Y{b}")
             for b in range(2)]
        VT = [[pers.tile([128, 130], BF16, tag=f"VT{b}_{j}", name=f"VT{b}_{j}")
               for j in range(8)] for b in range(2)]
        ocolsT = pers.tile([128, 1024, 2], BF16, tag="ocolsT", name="ocolsT")
        Ydiag = [[pers.tile([128, 1024], BF16, tag=f"Yd{h}_{b}",
                            name=f"Yd{h}_{b}") for b in range(2)]
                 for h in range(2)]
        Xdup = [[pers.tile([128, 1024], BF16, tag=f"Xd{h}_{a}",
                           name=f"Xd{h}_{a}") for a in range(2)]
                for h in range(2)]

        ident = pers.tile([128, 128], BF16, tag="ident", name="ident")
        nc.gpsimd.dma_start(ident[:], d["ident"][:])
        ones_sb = pers.tile([128, 2], BF16, tag="ones", name="ones")
        nc.gpsimd.dma_start(ones_sb[:], d["ones"][:])
        onesrow = pers.tile([1, 64], F32, tag="onesrow", name="onesrow")
        nc.gpsimd.dma_start(onesrow[:], d["onesrow"][:])
        wo_sb = pers.tile([128, H], BF16, tag="wo", name="wo")
        nc.gpsimd.dma_start(wo_sb[:], d["WoT"][:])
        bias = {}
        rowp = ctx.enter_context(tc.tile_pool(name="rowp", bufs=2))
        for bn in ("bqb", "bkb", "blinb", "bres2b", "bob"):
            row = rowp.tile([1, H], BF16, tag="rowst", name=f"row_{bn}")
            nc.gpsimd.dma_start(row[:], d[bn][:])
            bias[bn] = pers.tile([128, H], BF16, tag=bn, name=bn)
            nc.gpsimd.partition_broadcast(bias[bn][:], row[:])

        resg = [pers.tile([128, H], F32, tag=f"resg{lb}", name=f"resg{lb}")
                for lb in range(2)]
        res1_sb = [pers.tile([128, H], BF16, tag=f"r1s{lb}",
                             name=f"r1s{lb}") for lb in range(2)]
        res1T = [pers.tile([128, 256], BF16, tag=f"r1_{hb}",
                           name=f"r1_{hb}") for hb in range(8)]

        # ================= A: k-pass, VT seam, q-pass ====================
        with tc.tile_pool(name="phA", bufs=7) as pA, \
             tc.tile_pool(name="phN", bufs=1) as pN, \
             tc.tile_pool(name="psA", bufs=1, space="PSUM") as psA, \
             tc.tile_pool(name="psT", bufs=2, space="PSUM") as psT, \
             tc.tile_pool(name="pR", bufs=3) as pR, \
             tc.tile_pool(name="pW", bufs=2) as pW, \
             tc.tile_pool(name="pG", bufs=1) as pG, \
             tc.tile_pool(name="pAB", bufs=1) as pAB:
            nodeT_sb = []
            for g in range(8):
                t = pN.tile([128, 4, 256], BF16, tag=f"node{g}",
                            name=f"node{g}")
                nodeT_sb.append(t)
            nc.sync.dma_start(nodeT_sb[0][:], d["nodeT4"][0])

            def node_lhs(e, lb):
                return nodeT_sb[e // 4][:, e % 4, 128 * lb:128 * (lb + 1)]

            kps = [[psA.tile([128, 512], F32, tag=f"pj{lb}{ch}",
                             name=f"k{lb}{ch}") for ch in range(2)]
                   for lb in range(2)]
            for grp in range(16):
                wt = pA.tile([128, 2, H], BF16, tag="wk", name="wk")
                nc.sync.dma_start(wt[:], d["WkT16"][grp])
                if grp < 7:
                    nc.sync.dma_start(nodeT_sb[grp + 1][:],
                                      d["nodeT4"][grp + 1])
                for e2 in range(2):
                    e = 2 * grp + e2
                    st, sp = (e == 0), (e == 31)
                    for lb in range(2):
                        lhs = node_lhs(e, lb)
                        for ch in range(2):
                            nc.tensor.matmul(
                                kps[lb][ch][:], lhs,
                                wt[:, e2, 512 * ch:512 * (ch + 1)],
                                start=st, stop=sp)
            for lb in range(2):
                for ch in range(2):
                    sl = slice(512 * ch, 512 * (ch + 1))
                    nc.vector.tensor_add(Y[lb][:, sl], kps[lb][ch][:],
                                         bias["bkb"][:, sl])

            # V^T tiles + Ydiag builds fill the k->q seam
            for b0 in range(2):
                for j in range(8):
                    pt = psT.tile([128, 128], BF16, tag="tp", name="tp")
                    nc.tensor.transpose(pt[:], Y[b0][:, 128 * j:128 * (j + 1)],
                                        ident[:])
                    vt = VT[b0][j]
                    nc.vector.tensor_copy(vt[:, 0:64], pt[:, 0:64])
                    nc.vector.tensor_copy(vt[:, 64:65], ones_sb[:, 0:1])
                    nc.vector.tensor_copy(vt[:, 65:129], pt[:, 64:128])
                    nc.vector.tensor_copy(vt[:, 129:130], ones_sb[:, 1:2])
            for h in range(2):
                hp = slice(64 * h, 64 * (h + 1))
                for b0 in range(2):
                    yd = Ydiag[h][b0]
                    nc.vector.memzero(yd[:])
                    src = Y[b0][hp].rearrange("p (j two c) -> p j two c",
                                              two=2, c=64)
                    dst = yd[:].rearrange("p (j two c) -> p j two c",
                                          two=2, c=64)
                    nc.gpsimd.dma_start(dst[0:64, :, 0, :], src[:, :, 0, :])
                    nc.gpsimd.dma_start(dst[64:128, :, 1, :], src[:, :, 1, :])

            qps = [[psA.tile([128, 512], F32, tag=f"pj{lb}{ch}",
                             name=f"q{lb}{ch}") for ch in range(2)]
                   for lb in range(2)]
            for grp in range(16):
                wt = pA.tile([128, 2, H], BF16, tag="wq", name="wq")
                nc.sync.dma_start(wt[:], d["WqT16"][grp])
                for e2 in range(2):
                    e = 2 * grp + e2
                    st, sp = (e == 0), (e == 31)
                    for lb in range(2):
                        lhs = node_lhs(e, lb)
                        for ch in range(2):
                            nc.tensor.matmul(
                                qps[lb][ch][:], lhs,
                                wt[:, e2, 512 * ch:512 * (ch + 1)],
                                start=st, stop=sp)
            for a0 in range(2):
                for ch in range(2):
                    sl = slice(512 * ch, 512 * (ch + 1))
                    nc.vector.tensor_add(Xdup[0][a0][0:64, sl],
                                         qps[a0][ch][0:64, :],
                                         bias["bqb"][0:64, sl])
                    nc.vector.tensor_add(Xdup[1][a0][64:128, sl],
                                         qps[a0][ch][64:128, :],
                                         bias["bqb"][64:128, sl])
            for a0 in range(2):
                nc.gpsimd.dma_start(Xdup[0][a0][64:128, :], Xdup[0][a0][0:64, :])
                nc.gpsimd.dma_start(Xdup[1][a0][0:64, :], Xdup[1][a0][64:128, :])

            # ============= R: residual branch (hides in A's DMA) =========
            ab3 = pAB.tile([128, 16, 256], BF16, tag="ab3", name="ab3")
            nc.sync.dma_start(ab3[:], d["abT3"][:])
            rp1 = [[psA.tile([128, 512], F32, tag=f"pj{lb}{ch}",
                             name=f"rp1{lb}{ch}") for ch in range(2)]
                   for lb in range(2)]
            for t in range(16):
                wlt = pR.tile([128, H], BF16, tag="wlt", name="wlt")
                nc.sync.dma_start(wlt[:], d["WlinT"][128 * t:128 * (t + 1), :])
                for lb in range(2):
                    lhs = ab3[:, t, 128 * lb:128 * (lb + 1)]
                    for ch in range(2):
                        nc.tensor.matmul(rp1[lb][ch][:], lhs,
                                         wlt[:, 512 * ch:512 * (ch + 1)],
                                         start=(t == 0), stop=(t == 15))
            for lb in range(2):
                for ch in range(2):
                    sl = slice(512 * ch, 512 * (ch + 1))
                    nc.scalar.activation(res1_sb[lb][:, sl], rp1[lb][ch][:],
                                         AF.Identity)
            for hb in range(8):
                for lb in range(2):
                    tp = psT.tile([128, 128], BF16, tag="tp2", name="tp2")
                    nc.tensor.transpose(tp[:],
                                        res1_sb[lb][:, 128 * hb:128 * (hb + 1)],
                                        ident[:])
                    nc.scalar.activation(
                        res1T[hb][:, 128 * lb:128 * (lb + 1)], tp[:],
                        AF.Identity)
            rp2 = [[psA.tile([128, 512], F32, tag=f"pj{lb}{ch}",
                             name=f"rp2{lb}{ch}")
                    for ch in range(2)] for lb in range(2)]
            for wg in range(2):
                wr = pW.tile([128, 4, H], BF16, tag="wr", name="wr")
                nc.sync.dma_start(wr[:], d["WresT2"][wg])
                for h4 in range(4):
                    hb = 4 * wg + h4
                    for lb in range(2):
                        for ch in range(2):
                            nc.tensor.matmul(
                                rp2[lb][ch][:],
                                res1T[hb][:, 128 * lb:128 * (lb + 1)],
                                wr[:, h4, 512 * ch:512 * (ch + 1)],
                                start=(hb == 0), stop=(hb == 7))
            for lb in range(2):
                tt = pG.tile([128, H], F32, tag="tt", name="tt")
                for ch in range(2):
                    sl = slice(512 * ch, 512 * (ch + 1))
                    nc.vector.tensor_add(tt[:, sl], rp2[lb][ch][:],
                                         bias["bres2b"][:, sl])
                g = pG.tile([128, H], F32, tag=f"g{lb}", name=f"g{lb}")
                nc.scalar.activation(g[:], tt[:], AF.Sigmoid)
                nc.vector.tensor_add(resg[lb][:], res1_sb[lb][:],
                                     bias["blinb"][:])
                nc.vector.tensor_mul(resg[lb][:], resg[lb][:], g[:])
                nc.vector.tensor_add(resg[lb][:], resg[lb][:], bias["bob"][:])

        # ================= C: scrambled attention ========================
        rcp_t = [[pers.tile([1, 1024], F32, tag=f"rcp{a}{h}",
                            name=f"rcp{a}{h}") for h in range(2)]
                 for a in range(2)]
        o_sb = [[[pers.tile([65, 512], F32, tag=f"osb{a}{h}{ch}",
                            name=f"osb{a}{h}{ch}")
                  for ch in range(2)] for h in range(2)] for a in range(2)]
        with tc.tile_pool(name="pP", bufs=3) as pP, \
             tc.tile_pool(name="pM", bufs=2) as pM, \
             tc.tile_pool(name="psS", bufs=1, space="PSUM") as psS, \
             tc.tile_pool(name="psO", bufs=1, space="PSUM") as psO:
            for a0 in range(2):
                O_ps = [[psO.tile([65, 512], F32, tag=f"O{h}{ch}",
                                  name=f"O{h}{ch}")
                         for ch in range(2)] for h in range(2)]
                for b0 in range(2):
                    for j in range(8):
                        bt = 8 * b0 + j
                        s_ps = [psS.tile([128, 1024], F32, tag=f"s{h}",
                                         name=f"s{h}") for h in range(2)]
                        for h in range(2):
                            for ch in range(2):
                                nc.tensor.matmul(
                                    s_ps[h][:, 512 * ch:512 * (ch + 1)],
                                    Ydiag[h][b0][:, 128 * j:128 * (j + 1)],
                                    Xdup[h][a0][:, 512 * ch:512 * (ch + 1)],
                                    start=True, stop=True)
                        p_sb = [pP.tile([128, 1024], BF16, tag=f"p{h}",
                                        name=f"p{h}") for h in range(2)]
                        for h in range(2):
                            nc.scalar.activation(p_sb[h][:], s_ps[h][:],
                                                 AF.Exp, scale=0.125)
                        for h in range(2):
                            for ch in range(2):
                                nc.tensor.matmul(
                                    O_ps[h][ch][:],
                                    VT[b0][j][:, 65 * h:65 * (h + 1)],
                                    p_sb[h][:, 512 * ch:512 * (ch + 1)],
                                    start=(bt == 0), stop=(bt == 15))
                # denominators straight from psum rows -> reciprocal fast,
                # then psum release copies (DVE+ACT in parallel)
                for h in range(2):
                    r_sb = pM.tile([1, 1024], F32, tag="r", name="r")
                    for ch in range(2):
                        nc.vector.tensor_copy(r_sb[:, 512 * ch:512 * (ch + 1)],
                                              O_ps[h][ch][64:65, :])
                    nc.vector.reciprocal_approx_fast(rcp_t[a0][h][:], r_sb[:])
                for h in range(2):
                    for ch in range(2):
                        if ch == 0:
                            nc.vector.tensor_copy(o_sb[a0][h][ch][:],
                                                  O_ps[h][ch][:])
                        else:
                            nc.scalar.activation(o_sb[a0][h][ch][:],
                                                 O_ps[h][ch][:], AF.Identity)
                for h in range(2):
                    rcp = rcp_t[a0][h]
                    if a0 == 0:
                        rcpb = pM.tile([64, 1024], F32, tag="rcpb",
                                       name="rcpb")
                        nc.gpsimd.partition_broadcast(rcpb[:], rcp[:])
                        for ch in range(2):
                            nc.vector.tensor_mul(
                                ocolsT[64 * h:64 * (h + 1),
                                       512 * ch:512 * (ch + 1), a0],
                                o_sb[a0][h][ch][0:64, :],
                                rcpb[:, 512 * ch:512 * (ch + 1)])

        # ================= O: out-projection partial =====================
        with tc.tile_pool(name="pO", bufs=3) as pO, \
             tc.tile_pool(name="psF", bufs=2, space="PSUM") as psF:
            for h in range(2):
                for ch in range(2):
                    rb = psF.tile([64, 512], F32, tag="rb", name="rb")
                    nc.tensor.matmul(
                        rb[:], onesrow[:],
                        rcp_t[1][h][:, 512 * ch:512 * (ch + 1)],
                        start=True, stop=True)
                    nc.vector.tensor_mul(
                        ocolsT[64 * h:64 * (h + 1),
                               512 * ch:512 * (ch + 1), 1],
                        o_sb[1][h][ch][0:64, :], rb[:])
            oc_flat = ocolsT[:].rearrange("p a b -> p (a b)")
            for j in list(range(2, 16)) + [0, 1]:
                op = psF.tile([128, 1024], F32, tag="op", name="op",
                              bufs=3)
                for ch in range(2):
                    nc.tensor.matmul(op[:, 512 * ch:512 * (ch + 1)],
                                     oc_flat[:, 128 * j:128 * (j + 1)],
                                     wo_sb[:, 512 * ch:512 * (ch + 1)],
                                     start=True, stop=True)
                ob = pO.tile([128, H], F16, tag="ob", name="ob")
                if j < 2:
                    nc.vector.tensor_add(ob[:], op[:], resg[j][:])
                elif j % 2 == 0:
                    nc.vector.tensor_copy(ob[:], op[:])
                else:
                    nc.scalar.activation(ob[:], op[:], AF.Identity)
                nc.sync.dma_start(out[128 * j:128 * (j + 1), :], ob[:])


def _build_nc():
    nc = bacc.Bacc("TRN2", target_bir_lowering=False, debug=False,
                   num_devices=N_CORES)
    d = {}

    def din(name, shape, dt=BF16):
        d[name] = nc.dram_tensor(name, shape, dt, kind="ExternalInput").ap()

    din("nodeT4", (8, 128, 4, 256))
    din("WqT16", (16, 128, 2, H))
    din("WkT16", (16, 128, 2, H))
    din("abT3", (128, 16, 256))
    din("WlinT", (E2, H))
    din("WresT2", (2, 128, 4, H))
    din("WoT", (128, H))
    din("ident", (128, 128))
    din("ones", (128, 2))
    din("onesrow", (1, 64), F32)
    for bn in ("bqb", "bkb", "blinb", "bres2b", "bob"):
        din(bn, (1, H))
    out = nc.dram_tensor("out", (L, H), F16, kind="ExternalOutput").ap()
    with tile.TileContext(nc) as tc:
        _emit(nc, tc, d, out)
    nc.compile()
    return nc


def get_nc():
    if "nc" not in _NC_CACHE:
        _NC_CACHE["nc"] = _build_nc()
    return _NC_CACHE["nc"]


def build_in_maps(inputs):
    f32 = np.float32
    bf16 = ml_dtypes.bfloat16
    ne = np.asarray(inputs["node_embedding"], f32)
    ab = np.asarray(inputs["atten_bias"], f32)
    Wq = np.asarray(inputs["Wq"], f32)
    Wk = np.asarray(inputs["Wk"], f32)
    Wlin = np.asarray(inputs["Wlin"], f32)
    Wres = np.asarray(inputs["Wres"], f32)
    Wo = np.asarray(inputs["Wo"], f32)
    bq = np.asarray(inputs["bq"], f32)
    bk = np.asarray(inputs["bk"], f32)
    blin = np.asarray(inputs["blin"], f32)
    bres = np.asarray(inputs["bres"], f32)
    bo = np.asarray(inputs["bo"], f32)

    WkT16 = np.ascontiguousarray(
        Wk.T.reshape(16, 2, 128, H).transpose(0, 2, 1, 3)).astype(bf16)
    WlinT = np.ascontiguousarray(Wlin.T).astype(bf16)
    WresT2 = np.ascontiguousarray(
        Wres.T.reshape(2, 4, 128, H).transpose(0, 2, 1, 3)).astype(bf16)
    ident = np.eye(128, dtype=f32).astype(bf16)
    ones = np.ones((128, 2), f32).astype(bf16)
    bres2 = (Wres @ blin + bres).astype(f32)

    in_maps = []
    for c in range(N_CORES):
        rows = np.r_[128 * c:128 * (c + 1),
                     1024 + 128 * c:1024 + 128 * (c + 1)]
        colperm = np.concatenate([np.arange(64) + 64 * p for p in _perm16(c)])
        in_maps.append({
            "nodeT4": np.ascontiguousarray(
                ne[rows].T.reshape(8, 4, 128, 256).transpose(
                    0, 2, 1, 3)).astype(bf16),
            "WqT16": np.ascontiguousarray(
                Wq.T[:, colperm].reshape(16, 2, 128, H).transpose(
                    0, 2, 1, 3)).astype(bf16),
            "WkT16": WkT16,
            "abT3": np.ascontiguousarray(
                ab[rows].T.reshape(16, 128, 256).transpose(
                    1, 0, 2)).astype(bf16),
            "WlinT": WlinT,
            "WresT2": WresT2,
            "WoT": np.ascontiguousarray(
                Wo[:, 128 * c:128 * (c + 1)].T).astype(bf16),
            "ident": ident,
            "ones": ones,
            "onesrow": np.ones((1, 64), f32),
            "bqb": bq[colperm].reshape(1, H).astype(bf16),
            "bkb": bk.reshape(1, H).astype(bf16),
            "blinb": blin.reshape(1, H).astype(bf16),
            "bres2b": bres2.reshape(1, H).astype(bf16),
            "bob": bo.reshape(1, H).astype(bf16),
        })
    return in_maps


def combine_outputs(results):
    full = np.zeros((L, H), np.float32)
    for c in range(N_CORES):
        o = np.asarray(results[c]["out"], np.float32)
        perm = _perm16(c)
        for j in range(16):
            full[128 * perm[j]:128 * (perm[j] + 1)] += o[128 * j:128 * (j + 1)]
    return full


def kernel(**inputs):
    nc = get_nc()
    in_maps = build_in_maps(inputs)
    res = bass_utils.run_bass_kernel_spmd(nc, in_maps,
                                          core_ids=list(range(N_CORES)))
    return combine_outputs(res.results)



# revision 2
# speedup vs baseline: 1.0368x; 1.0368x over previous
"""Trainium2 Bass kernel for nn_BiasedMultiHeadAtten (8-core SPMD, tensor
parallel over heads).

The torch module's transpose(0,1)+reshape "scramble" means head n of the
attention only reads rows [64n,64n+64) u [1024+64n,1024+64n+64) of q/k, and
the per-head attention factors into four 1024x1024 score blocks with
contraction 64.  Sharding 2 heads per core therefore also shards the q/k
projections 8-way (256 of 2048 rows each).

Per core c (heads 2c, 2c+1):
  - q/k projections in fp8 (DoubleRow, contraction 256/step); weights are
    pre-scaled by 64 so they sit in fp8e4m3's normal range -- the scale is
    folded into the exp() scale and a 64-valued ones-column that rides in
    V^T to produce matched softmax denominators
  - scrambled attention in bf16: S^T = Y^T X per (b-block, query-chunk),
    exp on ACT (no max subtraction: |scores| bounded), AV via PE with the
    ones-column appended to V^T
  - out-proj partials + output DMA folded into the attention loop (per
    512-query chunk) so the PE/ACT/DMA all stream concurrently
  - gated-residual branch in bf16 for its 256 rows (tanh-form sigmoid so
    the scalar engine only ever loads one activation table set)
Host sums the 8 partial outputs with per-core row un-permutation.
"""

import numpy as np
import ml_dtypes

import concourse.bacc as bacc
import concourse.mybir as mybir
import concourse.tile as tile
from concourse import bass_utils

N_CORES = 8
L, H, E, E2, HD = 2048, 1024, 4096, 2048, 64
F32 = mybir.dt.float32
F16 = mybir.dt.float16
BF16 = mybir.dt.bfloat16
FP8 = mybir.dt.float8e4
AF = mybir.ActivationFunctionType
ALU = mybir.AluOpType
DR = mybir.MatmulPerfMode.DoubleRow

WSCALE = 64.0                      # q/k weight pre-scale for fp8 range
EXPSCALE = 0.125 / (WSCALE * WSCALE)

_NC_CACHE = {}


def _perm16(c):
    """Block permutation: device l-tile j holds global l-tile perm[j];
    perm[0] = c and perm[1] = 8 + c so the residual rows sit at tiles 0,1."""
    perm = list(range(16))

    def place(pos, val):
        i = perm.index(val)
        perm[pos], perm[i] = perm[i], perm[pos]

    place(0, c)
    place(1, 8 + c)
    return perm


def _emit(nc, tc, d, out):
    from contextlib import ExitStack

    with ExitStack() as ctx:
        pers = ctx.enter_context(tc.tile_pool(name="pers", bufs=1))

        # ---------------- persistent SBUF tiles ----------------
        Y = [pers.tile([128, H], BF16, tag=f"Y{b}", name=f"Y{b}")
             for b in range(2)]
        VT = [[pers.tile([128, 130], BF16, tag=f"VT{b}_{j}", name=f"VT{b}_{j}")
               for j in range(8)] for b in range(2)]
        ocolsT = pers.tile([128, 1024, 2], BF16, tag="ocolsT", name="ocolsT")
        Ydiag = [[pers.tile([128, 1024], BF16, tag=f"Yd{h}_{b}",
                            name=f"Yd{h}_{b}") for b in range(2)]
                 for h in range(2)]
        Xdup = [[pers.tile([128, 1024], BF16, tag=f"Xd{h}_{a}",
                           name=f"Xd{h}_{a}") for a in range(2)]
                for h in range(2)]

        # k/q fp8 operands (node: 2 tiles of 8 steps; weights stream in 4s)
        node_sb = [pers.tile([128, 8, 2, 256], FP8, tag=f"node{g}",
                             name=f"node{g}") for g in range(2)]
        nc.gpsimd.dma_start(node_sb[0][:], d["nodeDR"][0])
        nc.gpsimd.dma_start(node_sb[1][:], d["nodeDR"][1])

        ident = pers.tile([128, 128], BF16, tag="ident", name="ident")
        nc.gpsimd.dma_start(ident[:], d["ident"][:])
        ones_sb = pers.tile([128, 2], BF16, tag="ones", name="ones")
        nc.gpsimd.dma_start(ones_sb[:], d["ones64"][:])
        onesrow = pers.tile([1, 64], F32, tag="onesrow", name="onesrow")
        nc.gpsimd.dma_start(onesrow[:], d["onesrow"][:])
        wo_sb = pers.tile([128, H], BF16, tag="wo", name="wo")
        nc.gpsimd.dma_start(wo_sb[:], d["WoT"][:])
        bias = {}
        for bn in ("bqb", "bkb", "blinb", "bres2b", "bob"):
            bias[bn] = pers.tile([128, H], BF16, tag=bn, name=bn)
            nc.gpsimd.dma_start(bias[bn][:], d[bn][:])

        # residual-branch weights (bf16, pulled early on the scalar queue)
        ab3 = pers.tile([128, 16, 256], BF16, tag="ab3", name="ab3")
        wlin_sb = pers.tile([128, 16, H], BF16, tag="wlin", name="wlin")
        wres_sb = pers.tile([128, 8, H], BF16, tag="wres", name="wres")

        resg = [pers.tile([128, H], F32, tag=f"resg{lb}", name=f"resg{lb}")
                for lb in range(2)]
        res1_sb = [pers.tile([128, H], BF16, tag=f"r1s{lb}",
                             name=f"r1s{lb}") for lb in range(2)]
        res1T = [pers.tile([128, 256], BF16, tag=f"r1_{hb}",
                           name=f"r1_{hb}") for hb in range(8)]

        # pin the ACT table set (exp_and_others covers exp/tanh/identity)
        wz = pers.tile([128, 8], F32, tag="wz", name="wz")
        nc.vector.memset(wz[:], 0.0)
        we = pers.tile([128, 8], F32, tag="we", name="we")
        nc.scalar.activation(we[:], wz[:], AF.Exp)

        # ================= A: k-pass, VT seam, q-pass, residual ==========
        with tc.tile_pool(name="pW", bufs=3) as pW, \
             tc.tile_pool(name="psA", bufs=1, space="PSUM") as psA, \
             tc.tile_pool(name="psT", bufs=2, space="PSUM") as psT, \
             tc.tile_pool(name="pG", bufs=1) as pG:

            # R weights early on scalar-engine DMA queue (idle in phase A)
            nc.scalar.dma_start(ab3[:], d["abT3"][:])
            nc.scalar.dma_start(wlin_sb[:], d["WlinT3"][:])
            nc.scalar.dma_start(wres_sb[:], d["WresT3"][:])

            kps = [[psA.tile([128, 512], F32, tag=f"pj{lb}{ch}",
                             name=f"k{lb}{ch}") for ch in range(2)]
                   for lb in range(2)]
            for g4 in range(4):
                wt = pW.tile([128, 4, 2, H], FP8, tag="w", name="wk")
                nc.sync.dma_start(wt[:], d["WkDR"][g4])
                for s4 in range(4):
                    s = 4 * g4 + s4
                    st, sp = (s == 0), (s == 15)
                    for lb in range(2):
                        lhs = node_sb[s // 8][:, s % 8, :,
                                              128 * lb:128 * (lb + 1)]
                        for ch in range(2):
                            nc.tensor.matmul(
                                kps[lb][ch][:], lhs,
                                wt[:, s4, :, 512 * ch:512 * (ch + 1)],
                                start=st, stop=sp, perf_mode=DR)
            for lb in range(2):
                for ch in range(2):
                    sl = slice(512 * ch, 512 * (ch + 1))
                    nc.vector.tensor_add(Y[lb][:, sl], kps[lb][ch][:],
                                         bias["bkb"][:, sl])

            # V^T tiles + Ydiag builds fill the k->q seam
            for b0 in range(2):
                for j in range(8):
                    pt = psT.tile([128, 128], BF16, tag="tp", name="tp")
                    nc.tensor.transpose(pt[:], Y[b0][:, 128 * j:128 * (j + 1)],
                                        ident[:])
                    vt = VT[b0][j]
                    nc.vector.tensor_copy(vt[:, 0:64], pt[:, 0:64])
                    nc.vector.tensor_copy(vt[:, 64:65], ones_sb[:, 0:1])
                    nc.vector.tensor_copy(vt[:, 65:129], pt[:, 64:128])
                    nc.vector.tensor_copy(vt[:, 129:130], ones_sb[:, 1:2])
            for h in range(2):
                hp = slice(64 * h, 64 * (h + 1))
                for b0 in range(2):
                    yd = Ydiag[h][b0]
                    nc.vector.memzero(yd[:])
                    src = Y[b0][hp].rearrange("p (j two c) -> p j two c",
                                              two=2, c=64)
                    dst = yd[:].rearrange("p (j two c) -> p j two c",
                                          two=2, c=64)
                    nc.gpsimd.dma_start(dst[0:64, :, 0, :], src[:, :, 0, :])
                    nc.gpsimd.dma_start(dst[64:128, :, 1, :], src[:, :, 1, :])

            qps = [[psA.tile([128, 512], F32, tag=f"pj{lb}{ch}",
                             name=f"q{lb}{ch}") for ch in range(2)]
                   for lb in range(2)]
            for g4 in range(4):
                wt = pW.tile([128, 4, 2, H], FP8, tag="w", name="wq")
                nc.sync.dma_start(wt[:], d["WqDR"][g4])
                for s4 in range(4):
                    s = 4 * g4 + s4
                    st, sp = (s == 0), (s == 15)
                    for lb in range(2):
                        lhs = node_sb[s // 8][:, s % 8, :,
                                              128 * lb:128 * (lb + 1)]
                        for ch in range(2):
                            nc.tensor.matmul(
                                qps[lb][ch][:], lhs,
                                wt[:, s4, :, 512 * ch:512 * (ch + 1)],
                                start=st, stop=sp, perf_mode=DR)
            for a0 in range(2):
                for ch in range(2):
                    sl = slice(512 * ch, 512 * (ch + 1))
                    nc.vector.tensor_add(Xdup[0][a0][0:64, sl],
                                         qps[a0][ch][0:64, :],
                                         bias["bqb"][0:64, sl])
                    nc.vector.tensor_add(Xdup[1][a0][64:128, sl],
                                         qps[a0][ch][64:128, :],
                                         bias["bqb"][64:128, sl])
            for a0 in range(2):
                nc.gpsimd.dma_start(Xdup[0][a0][64:128, :],
                                    Xdup[0][a0][0:64, :])
                nc.gpsimd.dma_start(Xdup[1][a0][0:64, :],
                                    Xdup[1][a0][64:128, :])

            # ---- residual branch (bf16) ----
            rp1 = [[psA.tile([128, 512], F32, tag=f"pj{lb}{ch}",
                             name=f"rp1{lb}{ch}") for ch in range(2)]
                   for lb in range(2)]
            for t in range(16):
                for lb in range(2):
                    lhs = ab3[:, t, 128 * lb:128 * (lb + 1)]
                    for ch in range(2):
                        nc.tensor.matmul(rp1[lb][ch][:], lhs,
                                         wlin_sb[:, t, 512 * ch:512 * (ch + 1)],
                                         start=(t == 0), stop=(t == 15))
            for lb in range(2):
                for ch in range(2):
                    sl = slice(512 * ch, 512 * (ch + 1))
                    nc.scalar.activation(res1_sb[lb][:, sl], rp1[lb][ch][:],
                                         AF.Identity)
            for hb in range(8):
                for lb in range(2):
                    tp = psT.tile([128, 128], BF16, tag="tp2", name="tp2")
                    nc.tensor.transpose(tp[:],
                                        res1_sb[lb][:, 128 * hb:128 * (hb + 1)],
                                        ident[:])
                    nc.scalar.activation(
                        res1T[hb][:, 128 * lb:128 * (lb + 1)], tp[:],
                        AF.Identity)
            rp2 = [[psA.tile([128, 512], F32, tag=f"pj{lb}{ch}",
                             name=f"rp2{lb}{ch}")
                    for ch in range(2)] for lb in range(2)]
            for hb in range(8):
                for lb in range(2):
                    for ch in range(2):
                        nc.tensor.matmul(
                            rp2[lb][ch][:],
                            res1T[hb][:, 128 * lb:128 * (lb + 1)],
                            wres_sb[:, hb, 512 * ch:512 * (ch + 1)],
                            start=(hb == 0), stop=(hb == 7))
            for lb in range(2):
                tt = pG.tile([128, H], F32, tag="tt", name="tt")
                for ch in range(2):
                    sl = slice(512 * ch, 512 * (ch + 1))
                    nc.vector.tensor_add(tt[:, sl], rp2[lb][ch][:],
                                         bias["bres2b"][:, sl])
                g = pG.tile([128, H], F32, tag=f"g{lb}", name=f"g{lb}")
                # sigmoid(x) = 0.5*tanh(x/2) + 0.5 (stays in the exp table set)
                nc.scalar.activation(g[:], tt[:], AF.Tanh, scale=0.5)
                nc.vector.tensor_scalar(g[:], g[:], 0.5, 0.5,
                                        ALU.mult, ALU.add)
                nc.vector.tensor_add(resg[lb][:], res1_sb[lb][:],
                                     bias["blinb"][:])
                nc.vector.tensor_mul(resg[lb][:], resg[lb][:], g[:])
                nc.vector.tensor_add(resg[lb][:], resg[lb][:], bias["bob"][:])

        # ======= C: scrambled attention + folded out-projection ==========
        oc_flat = ocolsT[:].rearrange("p a b -> p (a b)")
        with tc.tile_pool(name="psS", bufs=2, space="PSUM") as psS, \
             tc.tile_pool(name="psO", bufs=1, space="PSUM") as psO, \
             tc.tile_pool(name="pP", bufs=3) as pP, \
             tc.tile_pool(name="pM", bufs=2) as pM, \
             tc.tile_pool(name="pOB", bufs=3) as pOB:

            def emit_out_tile(j):
                op = psS.tile([128, 1024], F32, tag="s", name=f"op{j}")
                for ch in range(2):
                    nc.tensor.matmul(op[:, 512 * ch:512 * (ch + 1)],
                                     oc_flat[:, 128 * j:128 * (j + 1)],
                                     wo_sb[:, 512 * ch:512 * (ch + 1)],
                                     start=True, stop=True)
                ob = pOB.tile([128, H], F16, tag="ob", name=f"ob{j}")
                if j < 2:
                    nc.vector.tensor_add(ob[:], op[:], resg[j][:])
                else:
                    nc.vector.tensor_copy(ob[:], op[:])
                nc.sync.dma_start(out[128 * j:128 * (j + 1), :], ob[:])

            pending = []
            for chq in range(2):
                cq = slice(512 * chq, 512 * (chq + 1))
                O_ps = [[psO.tile([65, 512], F32, tag=f"o{a0}{h}",
                                  name=f"O{a0}{h}") for h in range(2)]
                        for a0 in range(2)]
                for bt in range(16):
                    b0, jj = divmod(bt, 8)
                    for h in range(2):
                        s = psS.tile([128, 1024], F32, tag="s", name=f"s{h}")
                        for a0 in range(2):
                            nc.tensor.matmul(
                                s[:, 512 * a0:512 * (a0 + 1)],
                                Ydiag[h][b0][:, 128 * jj:128 * (jj + 1)],
                                Xdup[h][a0][:, cq], start=True, stop=True)
                        p = pP.tile([128, 1024], BF16, tag="p", name=f"p{h}")
                        nc.scalar.activation(p[:], s[:], AF.Exp,
                                             scale=EXPSCALE)
                        for a0 in range(2):
                            nc.tensor.matmul(
                                O_ps[a0][h][:],
                                VT[b0][jj][:, 65 * h:65 * (h + 1)],
                                p[:, 512 * a0:512 * (a0 + 1)],
                                start=(bt == 0), stop=(bt == 15))
                    if pending:
                        emit_out_tile(pending.pop(0))
                # normalize this query-chunk into ocolsT
                for h in range(2):
                    rcp = pM.tile([1, 1024], F32, tag="rcp", name="rcp")
                    for a0 in range(2):
                        nc.vector.tensor_copy(
                            rcp[:, 512 * a0:512 * (a0 + 1)],
                            O_ps[a0][h][64:65, :])
                    rcpf = pM.tile([1, 1024], F32, tag="rcpf", name="rcpf")
                    nc.vector.reciprocal_approx_fast(rcpf[:], rcp[:])
                    rb = psS.tile([64, 1024], F32, tag="s", name="rb")
                    for a0 in range(2):
                        nc.tensor.matmul(rb[:, 512 * a0:512 * (a0 + 1)],
                                         onesrow[:],
                                         rcpf[:, 512 * a0:512 * (a0 + 1)],
                                         start=True, stop=True)
                    rbs = pM.tile([64, 1024], F32, tag="rbs", name="rbs")
                    nc.vector.tensor_copy(rbs[:], rb[:])
                    for a0 in range(2):
                        nc.vector.tensor_mul(
                            ocolsT[64 * h:64 * (h + 1), cq, a0],
                            O_ps[a0][h][0:64, :],
                            rbs[0:64, 512 * a0:512 * (a0 + 1)])
                pending += [8 * chq + t for t in range(8)]
            while pending:
                emit_out_tile(pending.pop(0))


def _build_nc():
    nc = bacc.Bacc("TRN2", target_bir_lowering=False, debug=False,
                   num_devices=N_CORES)
    d = {}

    def din(name, shape, dt=BF16):
        d[name] = nc.dram_tensor(name, shape, dt, kind="ExternalInput").ap()

    din("nodeDR", (2, 128, 8, 2, 256), FP8)
    din("WqDR", (4, 128, 4, 2, H), FP8)
    din("WkDR", (4, 128, 4, 2, H), FP8)
    din("abT3", (128, 16, 256))
    din("WlinT3", (128, 16, H))
    din("WresT3", (128, 8, H))
    din("WoT", (128, H))
    din("ident", (128, 128))
    din("ones64", (128, 2))
    din("onesrow", (1, 64), F32)
    for bn in ("bqb", "bkb", "blinb", "bres2b", "bob"):
        din(bn, (128, H))
    out = nc.dram_tensor("out", (L, H), F16, kind="ExternalOutput").ap()
    with tile.TileContext(nc) as tc:
        _emit(nc, tc, d, out)
    nc.compile()
    return nc


def get_nc():
    if "nc" not in _NC_CACHE:
        _NC_CACHE["nc"] = _build_nc()
    return _NC_CACHE["nc"]


def build_in_maps(inputs):
    f32 = np.float32
    bf16 = ml_dtypes.bfloat16
    fp8 = mybir.dt.np(FP8)
    ne = np.asarray(inputs["node_embedding"], f32)
    ab = np.asarray(inputs["atten_bias"], f32)
    Wq = np.asarray(inputs["Wq"], f32)
    Wk = np.asarray(inputs["Wk"], f32)
    Wlin = np.asarray(inputs["Wlin"], f32)
    Wres = np.asarray(inputs["Wres"], f32)
    Wo = np.asarray(inputs["Wo"], f32)
    bq = np.asarray(inputs["bq"], f32)
    bk = np.asarray(inputs["bk"], f32)
    blin = np.asarray(inputs["blin"], f32)
    bres = np.asarray(inputs["bres"], f32)
    bo = np.asarray(inputs["bo"], f32)

    def dr_weights(WT):  # (E, H) -> (4, 128, 4, 2, H) fp8, pre-scaled
        return np.ascontiguousarray(
            (WT * WSCALE).reshape(4, 4, 2, 128, H).transpose(0, 3, 1, 2, 4)
        ).astype(fp8)

    WkDR = dr_weights(Wk.T)
    WlinT3 = np.ascontiguousarray(
        Wlin.T.reshape(16, 128, H).transpose(1, 0, 2)).astype(bf16)
    WresT3 = np.ascontiguousarray(
        Wres.T.reshape(8, 128, H).transpose(1, 0, 2)).astype(bf16)
    ident = np.eye(128, dtype=f32).astype(bf16)
    ones64 = np.full((128, 2), WSCALE, f32).astype(bf16)
    bres2 = (Wres @ blin + bres).astype(f32)

    def rep(x):
        return np.ascontiguousarray(
            np.broadcast_to(x.reshape(1, H), (128, H))).astype(bf16)

    in_maps = []
    for c in range(N_CORES):
        rows = np.r_[128 * c:128 * (c + 1),
                     1024 + 128 * c:1024 + 128 * (c + 1)]
        colperm = np.concatenate([np.arange(64) + 64 * p for p in _perm16(c)])
        in_maps.append({
            "nodeDR": np.ascontiguousarray(
                ne[rows].T.reshape(2, 8, 2, 128, 256).transpose(
                    0, 3, 1, 2, 4)).astype(fp8),
            "WqDR": dr_weights(Wq.T[:, colperm]),
            "WkDR": WkDR,
            "abT3": np.ascontiguousarray(
                ab[rows].T.reshape(16, 128, 256).transpose(
                    1, 0, 2)).astype(bf16),
            "WlinT3": WlinT3,
            "WresT3": WresT3,
            "WoT": np.ascontiguousarray(
                Wo[:, 128 * c:128 * (c + 1)].T).astype(bf16),
            "ident": ident,
            "ones64": ones64,
            "onesrow": np.ones((1, 64), f32),
            "bqb": rep(bq[colperm] * WSCALE),
            "bkb": rep(bk * WSCALE),
            "blinb": rep(blin),
            "bres2b": rep(bres2),
            "bob": rep(bo),
        })
    return in_maps


def combine_outputs(results):
    full = np.zeros((L, H), np.float32)
    for c in range(N_CORES):
        o = np.asarray(results[c]["out"], np.float32)
        perm = _perm16(c)
        for j in range(16):
            full[128 * perm[j]:128 * (perm[j] + 1)] += o[128 * j:128 * (j + 1)]
    return full


def kernel(**inputs):
    nc = get_nc()
    in_maps = build_in_maps(inputs)
    res = bass_utils.run_bass_kernel_spmd(nc, in_maps,
                                          core_ids=list(range(N_CORES)))
    return combine_outputs(res.results)
